# revision 3
# baseline (speedup 1.0000x reference)
"""AdaptiveGCN forward on 8 TRN2 NeuronCores (Bass/Tile).

Math (per the nn.Module reference):
  xr  = permute/reshape of x into (B*L, C, N)      [torch-faithful raw reshape]
  adp = softmax(relu(nodevec1 @ nodevec2), -1)
  out_list = [xr] + [xr@a^T, xr@a^T@a^T  for a in (a1, a2, adp)]
  o   = w @ concat(out_list, channel axis) + b     (1x1 conv)
  return o.reshape(B, L, N, C)                     [raw reshape]

Distribution: pure data-parallel over B (8 cores, 1 batch row each),
weights replicated, no collectives in forward.

Key layout fact (derived + numerically verified): per batch b the reference's
xr rows [b*L, (b+1)*L) are exactly  x[b].reshape(64, 65536).T.reshape(64, C, N).
Per output row m, T := xr[m].T (node-major, (N, C)) is reachable from the
contiguous slice x[b][:, 8m:8m+8, :] by partition-preserving strided copies:
  T[u_hi*64 + n_lo, k*128 + c_hi*16 + c_lo] = x[b][n_lo, 8m + c_hi, c_lo*8 + 2k + u_hi]

Order-2 diffusion uses (a^T)^2 = (a@a)^T so every concat member is a single
matmul from X: Y_j[m] = X_m @ P_j with P in {I, a1^T, (a1^2)^T, a2^T, (a2^2)^T,
adp^T, (adp^2)^T}. With lhsT = T-chunks (node-major) the PE emits Y_j[m]
channel-major in PSUM, which directly feeds the 1x1-conv matmuls
(lhsT = w^T chunks), accumulating all 7 concat members into one PSUM bank.
"""

import numpy as np

import concourse.bass as bass
import concourse.bacc as bacc
import concourse.mybir as mybir
import concourse.tile as tile
from concourse.bass_utils import run_bass_kernel_spmd
from concourse import masks

F32 = mybir.dt.float32
BF16 = mybir.dt.bfloat16

B, L, N, C = 8, 64, 512, 128
NK = N // 128          # 4 contraction chunks of 128
NJ = 7                 # concat members
GROUP = 4              # m's per DMA group
NG = L // GROUP        # 16 groups
AF = mybir.ActivationFunctionType
AX = mybir.AxisListType

_CACHE = {}


def build_graph():
    nc = bacc.Bacc("TRN2", target_bir_lowering=False, debug=False, num_devices=8)

    x_d = nc.declare_dram_parameter("x", [L, N, C], F32, isOutput=False)
    nv1t_d = nc.declare_dram_parameter("nv1t", [10, N], F32, isOutput=False)
    nv2_d = nc.declare_dram_parameter("nv2", [10, N], F32, isOutput=False)
    a1_d = nc.declare_dram_parameter("a1", [N, N], F32, isOutput=False)
    a1t_d = nc.declare_dram_parameter("a1t", [N, N], F32, isOutput=False)
    a2_d = nc.declare_dram_parameter("a2", [N, N], F32, isOutput=False)
    a2t_d = nc.declare_dram_parameter("a2t", [N, N], F32, isOutput=False)
    wt_d = nc.declare_dram_parameter("wt", [NJ * C, C], F32, isOutput=False)
    b_d = nc.declare_dram_parameter("bias", [C, 1], F32, isOutput=False)
    out_d = nc.declare_dram_parameter("out", [L, C, N], F32, isOutput=True)

    with tile.TileContext(nc) as tc:
        with (
            tc.tile_pool(name="const", bufs=1) as const,
            tc.tile_pool(name="setup", bufs=1) as setup,
            tc.tile_pool(name="smax", bufs=2) as smax,
            tc.tile_pool(name="sbig", bufs=3) as sbig_pool,
            tc.tile_pool(name="tcat", bufs=4) as tcat_pool,
            tc.tile_pool(name="ysb", bufs=12) as ysb_pool,
            tc.tile_pool(name="outsb", bufs=3) as outsb_pool,
            tc.tile_pool(name="ypsum", bufs=4, space=bass.MemorySpace.PSUM) as ypsum_pool,
            tc.tile_pool(name="opsum", bufs=2, space=bass.MemorySpace.PSUM) as opsum_pool,
        ):
            # ---------------- constants / weights ----------------
            # icat[u, k*512 + v] = 1.0 iff v == 128k + u   (bf16 identity chunks)
            icat = const.tile([128, NK * N], BF16, tag="icat")
            nc.gpsimd.memset(icat[:], 0.0)
            for k in range(NK):
                nc.gpsimd.affine_select(
                    out=icat[:, k * N:(k + 1) * N],
                    in_=icat[:, k * N:(k + 1) * N],
                    compare_op=mybir.AluOpType.not_equal,
                    fill=1.0,
                    base=128 * k,
                    pattern=[[-1, N]],
                    channel_multiplier=1,
                )

            # w^T chunks: wt_sb[c, j*128 + o] = w[o, j*128 + c]
            wt_sb = const.tile([C, NJ * C], BF16, tag="wt")
            nc.gpsimd.dma_start(
                out=wt_sb.rearrange("c (j o) -> c j o", j=NJ),
                in_=wt_d.ap().rearrange("(j c) o -> c j o", j=NJ),
            )
            b_sb = const.tile([C, 1], F32, tag="bsb")
            nc.sync.dma_start(out=b_sb[:], in_=b_d[:])

            # P_j chunk tiles, j=1..6; p_sb[j][k] = P_j[128k:128(k+1), :] (bf16)
            p_sb = {j: [const.tile([128, N], BF16, tag=f"p{j}_{k}", name=f"p{j}_{k}")
                        for k in range(NK)] for j in range(1, NJ)}
            # natural-orientation bf16 copies (lhsT for the squares)
            a1n = [setup.tile([128, N], BF16, tag=f"a1n{k}", name=f"a1n{k}") for k in range(NK)]
            a2n = [setup.tile([128, N], BF16, tag=f"a2n{k}", name=f"a2n{k}") for k in range(NK)]
            adpn = [setup.tile([128, N], BF16, tag=f"adpn{k}", name=f"adpn{k}") for k in range(NK)]
            for k in range(NK):
                sl = slice(128 * k, 128 * (k + 1))
                nc.gpsimd.dma_start(out=p_sb[1][k][:], in_=a1t_d[sl, :])
                nc.gpsimd.dma_start(out=p_sb[3][k][:], in_=a2t_d[sl, :])
                nc.gpsimd.dma_start(out=a1n[k][:], in_=a1_d[sl, :])
                nc.gpsimd.dma_start(out=a2n[k][:], in_=a2_d[sl, :])

            # ---------------- adaptive adjacency ----------------
            nv1t_sb = setup.tile([10, N], F32, tag="nv1t")
            nv2_sb = setup.tile([10, N], F32, tag="nv2")
            nc.sync.dma_start(out=nv1t_sb[:], in_=nv1t_d[:])
            nc.sync.dma_start(out=nv2_sb[:], in_=nv2_d[:])
            for r in range(NK):
                ep = ypsum_pool.tile([128, N], F32, tag="yp")
                nc.tensor.matmul(ep[:], nv1t_sb[:, 128 * r:128 * (r + 1)], nv2_sb[:],
                                 start=True, stop=True)
                es = smax.tile([128, N], F32, tag="es")
                nc.scalar.activation(es[:], ep[:], AF.Relu)
                negmx = smax.tile([128, 1], F32, tag="negmx")
                nc.vector.reduce_max(negmx[:], es[:], axis=AX.X, negate=True)
                pex = smax.tile([128, N], F32, tag="pex")
                nc.scalar.activation(pex[:], es[:], AF.Exp, bias=negmx[:], scale=1.0)
                sm = smax.tile([128, 1], F32, tag="sm")
                nc.vector.reduce_sum(sm[:], pex[:], axis=AX.X)
                rs = smax.tile([128, 1], F32, tag="rs")
                nc.vector.reciprocal(rs[:], sm[:])
                nc.vector.tensor_scalar_mul(adpn[r][:], pex[:], rs[:])

            # P5 = adp^T via identity matmuls
            for r in range(NK):
                pp = ypsum_pool.tile([128, N], F32, tag="yp")
                for k in range(NK):
                    nc.tensor.matmul(pp[:], adpn[k][:, 128 * r:128 * (r + 1)],
                                     icat[:, k * N:(k + 1) * N],
                                     start=(k == 0), stop=(k == NK - 1))
                nc.scalar.copy(p_sb[5][r][:], pp[:])

            # squares: P2 = P1@P1 (lhsT=a1 natural), P4, P6
            for nat, src_j, dst_j in ((a1n, 1, 2), (a2n, 3, 4), (adpn, 5, 6)):
                for r in range(NK):
                    pp = ypsum_pool.tile([128, N], F32, tag="yp")
                    for k in range(NK):
                        nc.tensor.matmul(pp[:], nat[k][:, 128 * r:128 * (r + 1)],
                                         p_sb[src_j][k][:],
                                         start=(k == 0), stop=(k == NK - 1))
                    nc.scalar.copy(p_sb[dst_j][r][:], pp[:])

            # ---------------- main loop ----------------
            for g in range(NG):
                sb = sbig_pool.tile([128, GROUP * 1024], F32, tag="sb")
                src = x_d[:, 8 * GROUP * g:8 * GROUP * (g + 1), :].rearrange("a b c -> a (b c)")
                # duplicate into both partition halves (copies are lane-local)
                nc.sync.dma_start(out=sb[0:64, :], in_=src)
                nc.sync.dma_start(out=sb[64:128, :], in_=src)
                out_tile = outsb_pool.tile([C, GROUP * N], F32, tag="ot")
                for t in range(GROUP):
                    m = GROUP * g + t
                    tcat = tcat_pool.tile([128, N], BF16, tag="tc")
                    smv = sb[:, t * 1024:(t + 1) * 1024].rearrange(
                        "p (ch cl nh) -> p nh ch cl", ch=8, cl=16, nh=8)
                    outv = tcat.rearrange("p (k ch cl) -> p k ch cl", k=NK, ch=8, cl=16)
                    nc.vector.tensor_copy(outv[0:64], smv[0:64, 0::2])
                    nc.vector.tensor_copy(outv[64:128], smv[64:128, 1::2])

                    y_sb = []
                    for j in range(NJ):
                        yp = ypsum_pool.tile([128, N], F32, tag="yp")
                        for k in range(NK):
                            rhs = (icat[:, k * N:(k + 1) * N] if j == 0
                                   else p_sb[j][k][:])
                            nc.tensor.matmul(yp[:], tcat[:, 128 * k:128 * (k + 1)], rhs,
                                             start=(k == 0), stop=(k == NK - 1))
                        ysb = ysb_pool.tile([128, N], BF16, tag="ys")
                        if j % 2 == 0:
                            nc.scalar.copy(ysb[:], yp[:])
                        else:
                            nc.vector.tensor_copy(ysb[:], yp[:])
                        y_sb.append(ysb)

                    op = opsum_pool.tile([C, N], F32, tag="op")
                    for j in range(NJ):
                        nc.tensor.matmul(op[:], wt_sb[:, C * j:C * (j + 1)], y_sb[j][:],
                                         start=(j == 0), stop=(j == NJ - 1))
                    # += bias, cast back to f32, stage for store
                    nc.scalar.activation(out_tile[:, N * t:N * (t + 1)], op[:],
                                         AF.Identity, bias=b_sb[:], scale=1.0)
                dst = out_d[GROUP * g:GROUP * (g + 1), :, :].rearrange("t o n -> o t n")
                nc.scalar.dma_start(
                    out=dst,
                    in_=out_tile.rearrange("o (t n) -> o t n", t=GROUP))

    nc.compile()
    return nc


def _get_compiled():
    if "nc" not in _CACHE:
        _CACHE["nc"] = build_graph()
    return _CACHE["nc"]


def make_in_maps(x, nodevec1, nodevec2, a1, a2, w, b):
    f = lambda a: np.ascontiguousarray(np.asarray(a), dtype=np.float32)
    shared = {
        "nv1t": f(np.asarray(nodevec1).T),
        "nv2": f(nodevec2),
        "a1": f(a1),
        "a1t": f(np.asarray(a1).T),
        "a2": f(a2),
        "a2t": f(np.asarray(a2).T),
        "wt": f(np.asarray(w).T),
        "bias": f(np.asarray(b).reshape(C, 1)),
    }
    return [dict(shared, x=f(np.asarray(x)[i])) for i in range(B)]


def kernel(x, nodevec1, nodevec2, a1, a2, w, b):
    nc = _get_compiled()
    in_maps = make_in_maps(x, nodevec1, nodevec2, a1, a2, w, b)
    res = run_bass_kernel_spmd(nc, in_maps, core_ids=list(range(B))).results
    out = np.concatenate([res[i]["out"] for i in range(B)], axis=0)  # (B*L, C, N)
    return out.reshape(B, L, N, C).astype(np.float32)


# revision 4
# speedup vs baseline: 1.0960x; 1.0960x over previous
"""AdaptiveGCN forward on 8 TRN2 NeuronCores (Bass/Tile).

Math (per the nn.Module reference):
  xr  = permute/reshape of x into (B*L, C, N)      [torch-faithful raw reshape]
  adp = softmax(relu(nodevec1 @ nodevec2), -1)
  out_list = [xr] + [xr@a^T, xr@a^T@a^T  for a in (a1, a2, adp)]
  o   = w @ concat(out_list, channel axis) + b     (1x1 conv)
  return o.reshape(B, L, N, C)                     [raw reshape]

Distribution: pure data-parallel over B (8 cores, 1 batch row each),
weights replicated, no collectives in forward.

Key layout fact (derived + numerically verified): per batch b the reference's
xr rows [b*L, (b+1)*L) are exactly  x[b].reshape(64, 65536).T.reshape(64, C, N).
Per output row m, T := xr[m].T (node-major, (N, C)) is reachable from the
contiguous slice x[b][:, 8m:8m+8, :] by partition-preserving strided copies:
  T[u_hi*64 + n_lo, k*128 + c_hi*16 + c_lo] = x[b][n_lo, 8m + c_hi, c_lo*8 + 2k + u_hi]
(the x slice is DMAed into both partition halves so the u_hi=1 copy stays
lane-local).

Order-2 diffusion uses (a^T)^2 = (a@a)^T so every concat member is a single
matmul from X: Y_j[m] = X_m @ P_j with P in {I, a1^T, (a1^2)^T, a2^T, (a2^2)^T,
adp^T, (adp^2)^T}. With lhsT = T-chunks (node-major) the PE emits Y_j[m]
channel-major in PSUM, which directly feeds the 1x1-conv matmuls
(lhsT = w^T chunks), accumulating all 7 concat members into one PSUM bank.
Y_0 (the identity member) is produced by PE transpose-mode (cheap N=128
passes) instead of a full identity matmul.
"""

import numpy as np

import concourse.bass as bass
import concourse.bacc as bacc
import concourse.mybir as mybir
import concourse.tile as tile
from concourse.bass_utils import run_bass_kernel_spmd

F32 = mybir.dt.float32
BF16 = mybir.dt.bfloat16

B, L, N, C = 8, 64, 512, 128
NK = N // 128          # 4 contraction chunks of 128
NJ = 7                 # concat members
GROUP = 4              # m's per DMA group
NG = L // GROUP        # 16 groups
AF = mybir.ActivationFunctionType
AX = mybir.AxisListType

_CACHE = {}


def build_graph():
    nc = bacc.Bacc("TRN2", target_bir_lowering=False, debug=False, num_devices=8)

    x_d = nc.declare_dram_parameter("x", [L, N, C], F32, isOutput=False)
    nv1t_d = nc.declare_dram_parameter("nv1t", [10, N], F32, isOutput=False)
    nv2_d = nc.declare_dram_parameter("nv2", [10, N], F32, isOutput=False)
    a1_d = nc.declare_dram_parameter("a1", [N, N], F32, isOutput=False)
    a1t_d = nc.declare_dram_parameter("a1t", [N, N], F32, isOutput=False)
    a2_d = nc.declare_dram_parameter("a2", [N, N], F32, isOutput=False)
    a2t_d = nc.declare_dram_parameter("a2t", [N, N], F32, isOutput=False)
    wt_d = nc.declare_dram_parameter("wt", [NJ * C, C], F32, isOutput=False)
    b_d = nc.declare_dram_parameter("bias", [C, 1], F32, isOutput=False)
    out_d = nc.declare_dram_parameter("out", [L, C, N], F32, isOutput=True)

    with tile.TileContext(nc) as tc:
        with (
            tc.tile_pool(name="const", bufs=1) as const,
            tc.tile_pool(name="setup", bufs=1) as setup,
            tc.tile_pool(name="smax", bufs=2) as smax,
            tc.tile_pool(name="sbig", bufs=3) as sbig_pool,
            tc.tile_pool(name="tcat", bufs=4) as tcat_pool,
            tc.tile_pool(name="ysb", bufs=12) as ysb_pool,
            tc.tile_pool(name="outsb", bufs=3) as outsb_pool,
            tc.tile_pool(name="ypsum", bufs=4, space=bass.MemorySpace.PSUM) as ypsum_pool,
            tc.tile_pool(name="y0psum", bufs=2, space=bass.MemorySpace.PSUM) as y0psum_pool,
            tc.tile_pool(name="opsum", bufs=2, space=bass.MemorySpace.PSUM) as opsum_pool,
        ):
            # ---------------- constants / weights ----------------
            i128 = const.tile([128, 128], BF16, tag="i128")
            nc.gpsimd.memset(i128[:], 0.0)
            nc.gpsimd.affine_select(
                out=i128[:], in_=i128[:],
                compare_op=mybir.AluOpType.not_equal, fill=1.0,
                base=0, pattern=[[-1, 128]], channel_multiplier=1,
            )

            # w^T chunks: wt_sb[c, j*128 + o] = w[o, j*128 + c]
            wt_sb = const.tile([C, NJ * C], BF16, tag="wt")
            nc.gpsimd.dma_start(
                out=wt_sb.rearrange("c (j o) -> c j o", j=NJ),
                in_=wt_d.ap().rearrange("(j c) o -> c j o", j=NJ),
            )
            b_sb = const.tile([C, 1], F32, tag="bsb")
            nc.sync.dma_start(out=b_sb[:], in_=b_d[:])

            # P_j tiles (128, NK*512): p_sb[j][:, k*512:(k+1)*512] = P_j rows 128k..
            p_sb = {j: const.tile([128, NK * N], BF16, tag=f"p{j}", name=f"p{j}")
                    for j in range(1, NJ)}
            # natural-orientation bf16 copies (lhsT for the squares)
            a1n = setup.tile([128, NK * N], BF16, tag="a1n")
            a2n = setup.tile([128, NK * N], BF16, tag="a2n")
            adpn = setup.tile([128, NK * N], BF16, tag="adpn")

            def load_chunked(dst, src_d):
                # dst[p, k*512 + v] = src[128k + p, v]; one SWDGE cast DMA
                nc.gpsimd.dma_start(
                    out=dst.rearrange("p (k v) -> p k v", k=NK),
                    in_=src_d.ap().rearrange("(k p) v -> p k v", k=NK),
                )

            load_chunked(p_sb[1], a1t_d)
            load_chunked(p_sb[3], a2t_d)
            load_chunked(a1n, a1_d)
            load_chunked(a2n, a2_d)

            nv1t_sb = setup.tile([10, N], F32, tag="nv1t")
            nv2_sb = setup.tile([10, N], F32, tag="nv2")
            nc.sync.dma_start(out=nv1t_sb[:], in_=nv1t_d[:])
            nc.sync.dma_start(out=nv2_sb[:], in_=nv2_d[:])

            # ---------------- adaptive adjacency (softmax chain) ----------------
            for r in range(NK):
                ep = ypsum_pool.tile([128, N], F32, tag="yp")
                nc.tensor.matmul(ep[:], nv1t_sb[:, 128 * r:128 * (r + 1)], nv2_sb[:],
                                 start=True, stop=True)
                es = smax.tile([128, N], F32, tag="es")
                nc.scalar.activation(es[:], ep[:], AF.Relu)
                negmx = smax.tile([128, 1], F32, tag="negmx")
                nc.vector.reduce_max(negmx[:], es[:], axis=AX.X, negate=True)
                pex = smax.tile([128, N], F32, tag="pex")
                nc.scalar.activation(pex[:], es[:], AF.Exp, bias=negmx[:], scale=1.0)
                sm = smax.tile([128, 1], F32, tag="sm")
                nc.vector.reduce_sum(sm[:], pex[:], axis=AX.X)
                rs = smax.tile([128, 1], F32, tag="rs")
                nc.vector.reciprocal(rs[:], sm[:])
                nc.vector.tensor_scalar_mul(adpn[:, r * N:(r + 1) * N], pex[:], rs[:])

            def square(nat, src_j, dst_j):
                # P_dst = P_src @ P_src, lhsT = natural-orientation chunks
                for r in range(NK):
                    pp = ypsum_pool.tile([128, N], F32, tag="yp")
                    for k in range(NK):
                        nc.tensor.matmul(
                            pp[:],
                            nat[:, k * N + 128 * r:k * N + 128 * (r + 1)],
                            p_sb[src_j][:, k * N:(k + 1) * N],
                            start=(k == 0), stop=(k == NK - 1))
                    nc.scalar.copy(p_sb[dst_j][:, r * N:(r + 1) * N], pp[:])

            # squares that don't need adp first (overlap the softmax chain)
            square(a1n, 1, 2)
            square(a2n, 3, 4)

            # P5 = adp^T via PE transpose-mode
            for r in range(NK):
                pp = y0psum_pool.tile([128, N], BF16, tag="y0p")
                for k in range(NK):
                    nc.tensor.matmul(
                        pp[:, 128 * k:128 * (k + 1)],
                        adpn[:, k * N + 128 * r:k * N + 128 * (r + 1)],
                        i128[:], is_transpose=True,
                        start=(k == 0), stop=(k == NK - 1))
                nc.scalar.copy(p_sb[5][:, r * N:(r + 1) * N], pp[:])

            square(adpn, 5, 6)

            # ---------------- main loop ----------------
            for g in range(NG):
                sb = sbig_pool.tile([128, GROUP * 1024], F32, tag="sb")
                src = x_d[:, 8 * GROUP * g:8 * GROUP * (g + 1), :].rearrange("a b c -> a (b c)")
                # duplicate into both partition halves (copies are lane-local)
                nc.sync.dma_start(out=sb[0:64, :], in_=src)
                nc.sync.dma_start(out=sb[64:128, :], in_=src)
                out_tile = outsb_pool.tile([C, GROUP * N], F32, tag="ot")
                for t in range(GROUP):
                    tcat = tcat_pool.tile([128, N], BF16, tag="tc")
                    smv = sb[:, t * 1024:(t + 1) * 1024].rearrange(
                        "p (ch cl nh) -> p nh ch cl", ch=8, cl=16, nh=8)
                    outv = tcat.rearrange("p (k ch cl) -> p k ch cl", k=NK, ch=8, cl=16)
                    nc.vector.tensor_copy(outv[0:64], smv[0:64, 0::2])
                    nc.vector.tensor_copy(outv[64:128], smv[64:128, 1::2])

                    y_sb = []
                    # j = 0: X_m itself, via transpose-mode (bf16 PSUM)
                    y0p = y0psum_pool.tile([128, N], BF16, tag="y0p")
                    for k in range(NK):
                        nc.tensor.matmul(
                            y0p[:, 128 * k:128 * (k + 1)],
                            tcat[:, 128 * k:128 * (k + 1)],
                            i128[:], is_transpose=True,
                            start=(k == 0), stop=(k == NK - 1))
                    y0sb = ysb_pool.tile([128, N], BF16, tag="ys")
                    nc.scalar.copy(y0sb[:], y0p[:])
                    y_sb.append(y0sb)

                    for j in range(1, NJ):
                        yp = ypsum_pool.tile([128, N], F32, tag="yp")
                        for k in range(NK):
                            nc.tensor.matmul(yp[:], tcat[:, 128 * k:128 * (k + 1)],
                                             p_sb[j][:, k * N:(k + 1) * N],
                                             start=(k == 0), stop=(k == NK - 1))
                        ysb = ysb_pool.tile([128, N], BF16, tag="ys")
                        if j % 2 == 0:
                            nc.scalar.copy(ysb[:], yp[:])
                        else:
                            nc.vector.tensor_copy(ysb[:], yp[:])
                        y_sb.append(ysb)

                    op = opsum_pool.tile([C, N], F32, tag="op")
                    for j in range(NJ):
                        nc.tensor.matmul(op[:], wt_sb[:, C * j:C * (j + 1)], y_sb[j][:],
                                         start=(j == 0), stop=(j == NJ - 1))
                    # += bias, cast back to f32, stage for store
                    nc.scalar.activation(out_tile[:, N * t:N * (t + 1)], op[:],
                                         AF.Identity, bias=b_sb[:], scale=1.0)
                dst = out_d[GROUP * g:GROUP * (g + 1), :, :].rearrange("t o n -> o t n")
                nc.scalar.dma_start(
                    out=dst,
                    in_=out_tile.rearrange("o (t n) -> o t n", t=GROUP))

    nc.compile()
    return nc


def _get_compiled():
    if "nc" not in _CACHE:
        _CACHE["nc"] = build_graph()
    return _CACHE["nc"]


def make_in_maps(x, nodevec1, nodevec2, a1, a2, w, b):
    f = lambda a: np.ascontiguousarray(np.asarray(a), dtype=np.float32)
    shared = {
        "nv1t": f(np.asarray(nodevec1).T),
        "nv2": f(nodevec2),
        "a1": f(a1),
        "a1t": f(np.asarray(a1).T),
        "a2": f(a2),
        "a2t": f(np.asarray(a2).T),
        "wt": f(np.asarray(w).T),
        "bias": f(np.asarray(b).reshape(C, 1)),
    }
    return [dict(shared, x=f(np.asarray(x)[i])) for i in range(B)]


def kernel(x, nodevec1, nodevec2, a1, a2, w, b):
    nc = _get_compiled()
    in_maps = make_in_maps(x, nodevec1, nodevec2, a1, a2, w, b)
    res = run_bass_kernel_spmd(nc, in_maps, core_ids=list(range(B))).results
    out = np.concatenate([res[i]["out"] for i in range(B)], axis=0)  # (B*L, C, N)
    return out.reshape(B, L, N, C).astype(np.float32)
